# revision 1
# baseline (speedup 1.0000x reference)
"""Trainium2 Bass kernel for the BDH fast-weight recurrent network.

Problem (see reference): for each batch element, a T=256-step recurrence with
  x_t   = L1norm(0.97*x_{t-1} + relu(v_t @ Dx^T))          (v_t = token_emb[idx_t])
  a*_t  = rho_{t-1} x_t ;  rho_t = 0.97*(rho_{t-1} + LN(v_t) x_t^T)
  y_t   = relu(LN(a*_t) @ Dy^T) * relu(x_t)
  out_t = LN(y_t @ E^T)

The kernel restructures this into feed-forward matmuls:
 - rho never materializes: a*_t = sum_{s<t} 0.97^{t-s} (x_s . x_t) LN(v_s)
   (decayed linear attention over the x sequence).
 - the x recurrence is linear given the per-step L1 scales S_t; since S_t ~ 100
   and eps=1e-6, S_t = sum(r_t) + 0.97 exactly in fp32, so
   X = G @ R with G[t,s] = 0.97^{t-s} / prod_{j=s..t} S_j, computed via
   exp/log with a log(100) shift for fp32 accuracy.

Sharding: data-parallel over batch, 4 sequences per NeuronCore x 8 cores,
no cross-core communication.
"""

import sys
import types

if "/opt/trn_rl_repo" not in sys.path:
    sys.path.insert(0, "/opt/trn_rl_repo")

import numpy as np

import concourse.bass as bass
import concourse.bacc as bacc
import concourse.tile as tile
from concourse import mybir
from concourse.bass_utils import run_bass_kernel_spmd

AF = mybir.ActivationFunctionType
OP = mybir.AluOpType

N, D, V = 4096, 256, 32000
B, T = 32, 256
BL = 4              # batch per core
NCORES = 8
XD = 0.97           # x decay
UD = 0.97           # rho decay
EPS = 1e-6
MU = float(np.log(100.0))
LNXD = float(np.log(XD))

F32 = mybir.dt.float32
MODE = "f16"                 # "f32" | "f32r" | "f16" for the large matmuls
MODE_DT = {"f32": mybir.dt.float32, "f32r": mybir.dt.float32r,
           "f16": mybir.dt.float16}
MM_DT = MODE_DT[MODE]
GT_LOG_SCALE = 8.0 * float(np.log(2.0))   # store GT * 2^8 (fp16 underflow guard)
GT_INV_SCALE = 1.0 / 256.0

NT = N // 128       # 32 n tiles
TT = T // 128       # 2 t tiles
DT = D // 128       # 2 d tiles


def _host_consts():
    """Constant tensors shipped to every core (computed in float64, cast f32)."""
    si = np.arange(T, dtype=np.float64)[:, None]
    ti = np.arange(T, dtype=np.float64)[None, :]
    k = ti - si
    kconst = np.where(k >= 0, k * LNXD - (k + 1) * MU + GT_LOG_SCALE, -200.0)
    kconst = kconst.astype(np.float32).reshape(TT, 128, T).transpose(1, 0, 2)
    decayT = np.where(k > 0, UD ** np.maximum(k, 0.0), 0.0)
    decayT = decayT.astype(np.float32).reshape(TT, 128, T).transpose(1, 0, 2)
    utones = (k >= 0).astype(np.float32).reshape(TT, 128, T).transpose(1, 0, 2)
    svb = np.full((T,), XD, np.float32)
    svb[0] = 0.0
    svb = svb.reshape(TT, 128).T.copy()
    return {
        "kconst": np.ascontiguousarray(kconst),   # (128, TT, T)
        "decayT": np.ascontiguousarray(decayT),   # (128, TT, T)
        "utones": np.ascontiguousarray(utones),   # (128, TT, T)
        "svb": np.ascontiguousarray(svb),         # (128, TT)
    }


def _ln_row(nc, tiny, z_in, out_ap, scratch_pool):
    """LayerNorm over the free dim (size D): out = (z - m)/(std_ddof1 + eps).

    z_in may be a PSUM or SBUF AP of shape (128, D). out_ap written f32/f32r.
    """
    sums = tiny.tile([128, 1], F32, tag="ln_sum")
    sq = tiny.tile([128, 1], F32, tag="ln_sq")
    scr = scratch_pool.tile([128, D], F32, tag="ln_scr")
    # scr = z*z (discarded), sq = sum(z*z); single-input op so z_in may be PSUM
    nc.scalar.activation(out=scr[:], in_=z_in, func=AF.Square, accum_out=sq[:])
    nc.vector.tensor_reduce(out=sums[:], in_=z_in, axis=mybir.AxisListType.X, op=OP.add)
    mean = tiny.tile([128, 1], F32, tag="ln_mean")
    nc.vector.tensor_scalar(
        out=mean[:], in0=sums[:], scalar1=1.0 / D, scalar2=None, op0=OP.mult,
    )
    # var_num = sq - D*mean^2  (= sum((z-m)^2))
    varn = tiny.tile([128, 1], F32, tag="ln_varn")
    nc.vector.tensor_scalar(
        out=varn[:], in0=mean[:], scalar1=mean[:], scalar2=float(-D), op0=OP.mult, op1=OP.mult,
    )
    nc.vector.tensor_tensor(out=varn[:], in0=varn[:], in1=sq[:], op=OP.add)
    # s = sqrt(varn/(D-1)); recip = 1/(s+eps)
    s = tiny.tile([128, 1], F32, tag="ln_s")
    nc.scalar.activation(out=s[:], in_=varn[:], func=AF.Sqrt, scale=1.0 / (D - 1))
    nc.vector.tensor_scalar(out=s[:], in0=s[:], scalar1=EPS, scalar2=None, op0=OP.add)
    recip = tiny.tile([128, 1], F32, tag="ln_recip")
    nc.vector.reciprocal(out=recip[:], in_=s[:])
    nc.vector.tensor_scalar(
        out=out_ap, in0=z_in, scalar1=mean[:], scalar2=recip[:],
        op0=OP.subtract, op1=OP.mult,
    )


def build_nc(mm_dt=MM_DT, dbg=False, dbg_keys=None):
    nc = bacc.Bacc("TRN2", target_bir_lowering=False, debug=False)
    dbg_specs = {
        "dbg_vprev": ([128, TT, D], F32), "dbg_U": ([128, TT, D], mm_dt),
        "dbg_R": ([128, TT, N], mm_dt), "dbg_lns": ([128, TT], F32),
        "dbg_GT": ([128, TT, T], mm_dt), "dbg_XT": ([128, NT, T], mm_dt),
        "dbg_AT": ([128, TT, T], mm_dt), "dbg_ynorm": ([128, TT, D], F32),
        "dbg_y": ([128, NT, T], mm_dt), "dbg_crow": ([1, T], F32),
    }
    dbg_d = {}
    if dbg:
        if dbg_keys is None:
            dbg_keys = set(dbg_specs)
        for k in dbg_keys:
            shp, dt_ = dbg_specs[k]
            dbg_d[k] = nc.dram_tensor(k, shp, dt_, kind="ExternalOutput").ap()

    def dump(key, tl):
        if dbg and key in dbg_d:
            nc.sync.dma_start(out=dbg_d[key][:], in_=tl)

    nc._dbg_tiles = {}

    idx_d = nc.dram_tensor("idx", [BL * T], mybir.dt.int32, kind="ExternalInput").ap()
    temb_d = nc.dram_tensor("temb", [V, D], F32, kind="ExternalInput").ap()
    dxt_d = nc.dram_tensor("dxt", [D, N], mm_dt, kind="ExternalInput").ap()
    dyt_d = nc.dram_tensor("dyt", [D, N], mm_dt, kind="ExternalInput").ap()
    et_d = nc.dram_tensor("et", [N, D], mm_dt, kind="ExternalInput").ap()
    kconst_d = nc.dram_tensor("kconst", [128, TT, T], F32, kind="ExternalInput").ap()
    decayT_d = nc.dram_tensor("decayT", [128, TT, T], F32, kind="ExternalInput").ap()
    utones_d = nc.dram_tensor("utones", [128, TT, T], F32, kind="ExternalInput").ap()
    svb_d = nc.dram_tensor("svb", [128, TT], F32, kind="ExternalInput").ap()
    out_d = nc.dram_tensor("out", [BL, T, D], F32, kind="ExternalOutput").ap()

    with tile.TileContext(nc) as tc:
        with (
            tc.tile_pool(name="consts", bufs=1) as consts,
            tc.tile_pool(name="xt", bufs=1) as xtp,
            tc.tile_pool(name="big", bufs=2) as big,
            tc.tile_pool(name="mid", bufs=2) as mid,
            tc.tile_pool(name="tiny", bufs=4) as tiny,
            tc.tile_pool(name="scratch", bufs=4) as scratch,
            tc.tile_pool(name="ps", bufs=8, space="PSUM") as ps,
        ):
            # ---- resident constants ----
            dxt = consts.tile([128, DT, N], mm_dt)
            nc.sync.dma_start(out=dxt[:], in_=dxt_d.rearrange("(k p) n -> p k n", p=128))
            dyt = consts.tile([128, DT, N], mm_dt)
            nc.sync.dma_start(out=dyt[:], in_=dyt_d.rearrange("(k p) n -> p k n", p=128))
            kconst = consts.tile([128, TT, T], F32)
            nc.sync.dma_start(out=kconst[:], in_=kconst_d[:])
            decayT = consts.tile([128, TT, T], F32)
            nc.sync.dma_start(out=decayT[:], in_=decayT_d[:])
            utones = consts.tile([128, TT, T], F32)
            nc.sync.dma_start(out=utones[:], in_=utones_d[:])
            svb = consts.tile([128, TT], F32)
            nc.sync.dma_start(out=svb[:], in_=svb_d[:])
            idx_t = consts.tile([128, 2 * BL], mybir.dt.int32)
            nc.sync.dma_start(out=idx_t[:], in_=idx_d.rearrange("(j p) -> p j", p=128))
            ident = consts.tile([128, 128], F32)
            from concourse.masks import make_identity
            make_identity(nc, ident[:])
            ones1 = consts.tile([1, 128], F32)
            nc.vector.memset(ones1[:], 1.0)

            for b in range(BL):
                # ---- embedding gather + transpose + U = LN(vprev) ----
                vprev = mid.tile([128, TT, D], F32, tag="vprev")
                vprevT = mid.tile([128, DT, T], mm_dt, tag="vprevT")
                U = mid.tile([128, TT, D], mm_dt, tag="U")
                for m in range(TT):
                    nc.gpsimd.indirect_dma_start(
                        out=vprev[:, m, :],
                        out_offset=None,
                        in_=temb_d[:],
                        in_offset=bass.IndirectOffsetOnAxis(
                            ap=idx_t[:, TT * b + m : TT * b + m + 1], axis=0
                        ),
                    )
                    for kd in range(DT):
                        pt = ps.tile([128, 128], F32, tag="ps")
                        nc.tensor.transpose(
                            out=pt[:], in_=vprev[:, m, kd * 128 : (kd + 1) * 128],
                            identity=ident[:],
                        )
                        nc.vector.tensor_copy(
                            out=vprevT[:, kd, m * 128 : (m + 1) * 128], in_=pt[:]
                        )
                    _ln_row(nc, tiny, vprev[:, m, :], U[:, m, :], scratch)

                if b == 0:
                    dump("dbg_vprev", vprev[:])
                    dump("dbg_U", U[:])

                # ---- R = relu(vprev @ Dx^T), with row sums ----
                R = big.tile([128, TT, N], mm_dt, tag="R")
                rs = tiny.tile([128, TT, 8], F32, tag="rs")
                for m in range(TT):
                    for n in range(8):
                        pr = ps.tile([128, 512], F32, tag="ps")
                        for kd in range(DT):
                            nc.tensor.matmul(
                                pr[:],
                                vprevT[:, kd, m * 128 : (m + 1) * 128],
                                dxt[:, kd, n * 512 : (n + 1) * 512],
                                start=(kd == 0),
                                stop=(kd == DT - 1),
                            )
                        nc.scalar.activation(
                            out=R[:, m, n * 512 : (n + 1) * 512], in_=pr[:],
                            func=AF.Relu, accum_out=rs[:, m, n : n + 1],
                        )

                if b == 0:
                    dump("dbg_R", R[:])

                # ---- G matrix (transposed): GT[s,t] = exp(kconst + C'_{s-1} - C'_t) ----
                lns = tiny.tile([128, TT], F32, tag="lns")
                for m in range(TT):
                    rsum = tiny.tile([128, 1], F32, tag="rsum")
                    nc.vector.tensor_reduce(
                        out=rsum[:], in_=rs[:, m, :], axis=mybir.AxisListType.X, op=OP.add
                    )
                    # ln(Rsum + svb) - MU
                    nc.scalar.activation(
                        out=lns[:, m : m + 1], in_=rsum[:], func=AF.Ln,
                        bias=svb[:, m : m + 1],
                    )
                    nc.vector.tensor_scalar(
                        out=lns[:, m : m + 1], in0=lns[:, m : m + 1],
                        scalar1=-MU, scalar2=None, op0=OP.add,
                    )
                # C' row (1, T): cumsum over t via upper-tri ones
                pcrow = ps.tile([1, T], F32, tag="ps")
                for j in range(TT):
                    nc.tensor.matmul(
                        pcrow[:], lns[:, j : j + 1], utones[:, j, :],
                        start=(j == 0), stop=(j == TT - 1),
                    )
                crow = tiny.tile([1, T], F32, tag="crow")
                nc.vector.tensor_copy(out=crow[:], in_=pcrow[:])
                # broadcast C' row to 128 partitions
                pbcast = ps.tile([128, T], F32, tag="ps")
                nc.tensor.matmul(pbcast[:], ones1[:], crow[:], start=True, stop=True)
                GT = mid.tile([128, TT, T], mm_dt, tag="GT")
                for m in range(TT):
                    # C' col for this s-tile, then csm1 = C' - lns
                    pccol = ps.tile([128, 1], F32, tag="ps")
                    for j in range(m + 1):
                        nc.tensor.matmul(
                            pccol[:], utones[:, j, m * 128 : (m + 1) * 128],
                            lns[:, j : j + 1],
                            start=(j == 0), stop=(j == m),
                        )
                    csm1 = tiny.tile([128, 1], F32, tag="csm1")
                    nc.vector.tensor_scalar(
                        out=csm1[:], in0=pccol[:], scalar1=lns[:, m : m + 1],
                        scalar2=None, op0=OP.subtract,
                    )
                    tmp = scratch.tile([128, T], F32, tag="gt_tmp")
                    nc.vector.tensor_tensor(
                        out=tmp[:], in0=kconst[:, m, :], in1=pbcast[:], op=OP.subtract
                    )
                    nc.scalar.activation(
                        out=GT[:, m, :], in_=tmp[:], func=AF.Exp, bias=csm1[:],
                    )

                if b == 0:
                    dump("dbg_lns", lns[:])
                    dump("dbg_crow", crow[:])
                    dump("dbg_GT", GT[:])

                # ---- X^T = R^T @ G^T  (n on partitions, t free) ----
                XT = xtp.tile([128, NT, T], mm_dt, tag="XT")
                for _nm, _tl in [("vprev", vprev), ("vprevT", vprevT), ("U", U),
                                 ("R", R), ("lns", lns), ("GT", GT), ("XT", XT)]:
                    nc._dbg_tiles[(_nm, b)] = _tl
                for nt in range(NT):
                    px = ps.tile([128, T], F32, tag="ps")
                    for k in range(TT):
                        nc.tensor.matmul(
                            px[:], R[:, k, nt * 128 : (nt + 1) * 128], GT[:, k, :],
                            start=(k == 0), stop=(k == TT - 1),
                        )
                    nc.vector.tensor_scalar(
                        out=XT[:, nt, :], in0=px[:], scalar1=GT_INV_SCALE,
                        scalar2=None, op0=OP.mult,
                    )

                if b == 0:
                    dump("dbg_XT", XT[:])

                # ---- scores S = X X^T, masked/decayed -> AT (s part, t free) ----
                AT = mid.tile([128, TT, T], mm_dt, tag="AT")
                for st in range(TT):
                    psc = ps.tile([128, T], F32, tag="ps")
                    for k in range(NT):
                        nc.tensor.matmul(
                            psc[:], XT[:, k, st * 128 : (st + 1) * 128], XT[:, k, :],
                            start=(k == 0), stop=(k == NT - 1),
                        )
                    nc.vector.tensor_tensor(
                        out=AT[:, st, :], in0=psc[:], in1=decayT[:, st, :], op=OP.mult
                    )

                if b == 0:
                    dump("dbg_AT", AT[:])

                # ---- a* = AT^T @ U ; ynorm = LN(a*) ; transpose -> ynormT ----
                ynorm = mid.tile([128, TT, D], F32, tag="ynorm")
                ynormT = mid.tile([128, DT, T], mm_dt, tag="ynormT")
                nc._dbg_tiles[("AT", b)] = AT
                nc._dbg_tiles[("ynorm", b)] = ynorm
                nc._dbg_tiles[("ynormT", b)] = ynormT
                for tt in range(TT):
                    pa = ps.tile([128, D], F32, tag="ps")
                    for k in range(tt + 1):
                        nc.tensor.matmul(
                            pa[:], AT[:, k, tt * 128 : (tt + 1) * 128], U[:, k, :],
                            start=(k == 0), stop=(k == tt),
                        )
                    _ln_row(nc, tiny, pa[:], ynorm[:, tt, :], scratch)
                    for kd in range(DT):
                        pt = ps.tile([128, 128], F32, tag="ps")
                        nc.tensor.transpose(
                            out=pt[:], in_=ynorm[:, tt, kd * 128 : (kd + 1) * 128],
                            identity=ident[:],
                        )
                        nc.vector.tensor_copy(
                            out=ynormT[:, kd, tt * 128 : (tt + 1) * 128], in_=pt[:]
                        )

                # ---- y^T = relu(Dy @ ynorm^T) * X^T  (in-place into XT) ----
                for nt in range(NT):
                    py = ps.tile([128, T], F32, tag="ps")
                    for kd in range(DT):
                        nc.tensor.matmul(
                            py[:], dyt[:, kd, nt * 128 : (nt + 1) * 128],
                            ynormT[:, kd, :],
                            start=(kd == 0), stop=(kd == DT - 1),
                        )
                    yr = scratch.tile([128, T], mm_dt, tag="yrelu")
                    nc.scalar.activation(out=yr[:], in_=py[:], func=AF.Relu)
                    nc.vector.tensor_tensor(
                        out=XT[:, nt, :], in0=yr[:], in1=XT[:, nt, :], op=OP.mult
                    )

                if b == 0:
                    dump("dbg_ynorm", ynorm[:])
                    dump("dbg_y", XT[:])

                # ---- v* = LN(y @ E^T) -> out ----
                et = big.tile([128, NT, D], mm_dt, tag="et")
                nc.sync.dma_start(out=et[:], in_=et_d.rearrange("(k p) d -> p k d", p=128))
                for tt in range(TT):
                    pv = ps.tile([128, D], F32, tag="ps")
                    for k in range(NT):
                        nc.tensor.matmul(
                            pv[:], XT[:, k, tt * 128 : (tt + 1) * 128], et[:, k, :],
                            start=(k == 0), stop=(k == NT - 1),
                        )
                    vstar = scratch.tile([128, D], F32, tag="vstar")
                    _ln_row(nc, tiny, pv[:], vstar[:], scratch)
                    nc.sync.dma_start(
                        out=out_d[b, tt * 128 : (tt + 1) * 128, :], in_=vstar[:]
                    )

    nc.compile()
    return nc


_NC_CACHE = {}


def _get_nc(mm_dt=MM_DT):
    key = str(mm_dt)
    if key not in _NC_CACHE:
        _NC_CACHE[key] = build_nc(mm_dt)
    return _NC_CACHE[key]


def kernel(idx, token_emb, E, Dx, Dy):
    wdt = mybir.dt.np(MM_DT)
    idx = np.ascontiguousarray(np.asarray(idx).astype(np.int32))
    token_emb = np.ascontiguousarray(np.asarray(token_emb, np.float32))
    dxt = np.ascontiguousarray(np.asarray(Dx, np.float32).T.astype(wdt))
    dyt = np.ascontiguousarray(np.asarray(Dy, np.float32).T.astype(wdt))
    et = np.ascontiguousarray(np.asarray(E, np.float32).T.astype(wdt))
    consts = _host_consts()

    nc = _get_nc()
    shared = {
        "temb": token_emb, "dxt": dxt, "dyt": dyt, "et": et, **consts,
    }
    in_maps = []
    for c in range(NCORES):
        m = dict(shared)
        m["idx"] = np.ascontiguousarray(idx[c * BL : (c + 1) * BL].reshape(-1))
        in_maps.append(m)

    res = run_bass_kernel_spmd(nc, in_maps, core_ids=list(range(NCORES)))
    out = np.concatenate([r["out"] for r in res.results], axis=0)
    return out

